# revision 5
# baseline (speedup 1.0000x reference)
"""BiEncoder (bidirectional LSTM over video features) Trainium2 kernel.

Sharding: 8 NeuronCores = 4 batch groups (B=64 each) x 2 directions.
Every core runs the SAME program (SPMD); the host hands backward-direction
cores time-reversed inputs and the direction's weights, and un-reverses the
outputs.

Per-core program:
  phase A (per 8-step chunk): embed  v = video @ W_e.T + b_e   (fp16 matmul)
                              xg     = v @ W_ih_s.T + b_s      (fp16 matmul)
  phase B (scan, 64 steps):   hg     = W_hh_s @ h_prev         (fp16 matmul)
                              t      = tanh(hg + xg)           (one ACT op)
                              c      = (t_f+1)/2*c + (t_i+1)/2*t_g
                              h      = (t_o+1)/2*tanh(c)
  using sigmoid(x) = (tanh(x/2)+1)/2 with the 1/2 folded into the i/f/o
  rows of W_ih/W_hh/bias on the host, so ONE tanh instruction covers all
  four gate groups.
"""

import sys
import time

for _p in ("/opt/trn_rl_repo", "/root/.axon_site/_ro/trn_rl_repo"):
    if _p not in sys.path:
        sys.path.insert(0, _p)

import numpy as np

import concourse.tile as tile
from concourse import bacc, mybir
from concourse.bass import ts
from concourse.bass_utils import run_bass_kernel_spmd

F16 = mybir.dt.float16
F32 = mybir.dt.float32
AF = mybir.ActivationFunctionType
OP = mybir.AluOpType

B, T, F, P, H = 256, 64, 2048, 512, 512
NB = 4          # batch groups
BC = B // NB    # 64 per-core batch
TC = 8          # timesteps per phase-A chunk
NCHUNK = T // TC
KF = F // 128   # 16  F tiles
KP = P // 128   # 4   P tiles
KH = H // 128   # 4   H tiles
MG = 4 * H // 128  # 16 gate tiles


def build_nc():
    nc = bacc.Bacc("TRN2", target_bir_lowering=False, debug=False, num_devices=8)

    vt_d = nc.dram_tensor("vt", [KF, 128, T, BC], F16, kind="ExternalInput")
    wet_d = nc.dram_tensor("w_et", [KF, 128, P], F16, kind="ExternalInput")
    bet_d = nc.dram_tensor("b_e_t", [128, KP], F32, kind="ExternalInput")
    wih_d = nc.dram_tensor("w_iht", [KP, 128, 4 * H], F16, kind="ExternalInput")
    whh_d = nc.dram_tensor("w_hht", [KH, 128, 4 * H], F16, kind="ExternalInput")
    bias_d = nc.dram_tensor("bias", [128, MG], F32, kind="ExternalInput")
    out_d = nc.dram_tensor("out_h", [T, 128, KH, BC], F16, kind="ExternalOutput")

    with tile.TileContext(nc) as tc:
        with (
            tc.tile_pool(name="const", bufs=1) as const,
            tc.tile_pool(name="vload", bufs=2) as vload,
            tc.tile_pool(name="vtp", bufs=2) as vtp,
            tc.tile_pool(name="xchunk", bufs=2) as xchunk,
            tc.tile_pool(name="state", bufs=3) as state,
            tc.tile_pool(name="tmp", bufs=2) as tmp,
            tc.tile_pool(name="psv", bufs=2, space="PSUM") as psv,
            tc.tile_pool(name="psx", bufs=2, space="PSUM") as psx,
            tc.tile_pool(name="psg", bufs=2, space="PSUM") as psg,
        ):
            # resident weights
            wet = const.tile([128, KF, P], F16)
            nc.sync.dma_start(wet[:], wet_d.ap().rearrange("ko p m -> p ko m"))
            wih = const.tile([128, KP, 4 * H], F16)
            nc.sync.dma_start(wih[:], wih_d.ap().rearrange("kp p g -> p kp g"))
            whh = const.tile([128, KH, 4 * H], F16)
            nc.sync.dma_start(whh[:], whh_d.ap().rearrange("kh p g -> p kh g"))
            bet = const.tile([128, KP], F32)
            nc.sync.dma_start(bet[:], bet_d.ap())
            bias = const.tile([128, MG], F32)
            nc.sync.dma_start(bias[:], bias_d.ap())

            # initial state
            h_prev = state.tile([128, KH, BC], F16, tag="h")
            nc.gpsimd.memset(h_prev[:], 0.0)
            c_prev = state.tile([128, KH, BC], F32, tag="c")
            nc.gpsimd.memset(c_prev[:], 0.0)

            xg_chunks = []

            def phase_a(c):
                vch = vload.tile([128, KF, TC * BC], F16, tag="vch")
                nc.sync.dma_start(
                    vch[:],
                    vt_d.ap()[:, :, c * TC : (c + 1) * TC, :].rearrange(
                        "ko p t b -> p ko (t b)"
                    ),
                )
                vsb = vtp.tile([128, KP, TC * BC], F16, tag="vsb")
                for mp in range(KP):
                    pv = psv.tile([128, TC * BC], F32, tag="pv")
                    for ko in range(KF):
                        nc.tensor.matmul(
                            pv[:],
                            wet[:, ko, ts(mp, 128)],
                            vch[:, ko, :],
                            start=(ko == 0),
                            stop=(ko == KF - 1),
                        )
                    # v = psum + b_e   (fp16 out)
                    nc.vector.tensor_scalar(
                        vsb[:, mp, :], pv[:], bet[:, mp : mp + 1], None, OP.add
                    )
                xgc = xchunk.tile([128, TC, MG, BC], F16, tag="xgc")
                for m in range(MG):
                    px = psx.tile([128, TC, BC], F32, tag="px")
                    # nh outer: accumulation groups must not interleave within
                    # a PSUM bank (start=True clears has_written bank-wide)
                    for nh in range(2):  # N=256 pieces
                        for kp in range(KP):
                            nc.tensor.matmul(
                                px[:, nh * (TC // 2) : (nh + 1) * (TC // 2), :],
                                wih[:, kp, ts(m, 128)],
                                vsb[:, kp, nh * (TC * BC // 2) : (nh + 1) * (TC * BC // 2)],
                                start=(kp == 0),
                                stop=(kp == KP - 1),
                            )
                    # xg = psum + bias  (fp16 out, strided over t)
                    nc.vector.tensor_scalar(
                        xgc[:, :, m, :], px[:], bias[:, m : m + 1], None, OP.add
                    )
                return xgc

            def scan_step(t, xgc, tl):
                nonlocal h_prev, c_prev
                pg = psg.tile([128, MG, BC], F32, tag="pg")
                for m in range(MG):
                    for kh in range(KH):
                        nc.tensor.matmul(
                            pg[:, m, :],
                            whh[:, kh, ts(m, 128)],
                            h_prev[:, kh, :],
                            start=(kh == 0),
                            stop=(kh == KH - 1),
                        )
                gs = tmp.tile([128, MG, BC], F16, tag="gs")
                nc.vector.tensor_tensor(gs[:], pg[:], xgc[:, tl, :, :], OP.add)
                th = tmp.tile([128, MG, BC], F16, tag="th")
                nc.scalar.activation(th[:], gs[:], AF.Tanh)
                t_i = th[:, 0 * KH : 1 * KH, :]
                t_f = th[:, 1 * KH : 2 * KH, :]
                t_g = th[:, 2 * KH : 3 * KH, :]
                t_o = th[:, 3 * KH : 4 * KH, :]
                uf = tmp.tile([128, KH, BC], F32, tag="uf")
                nc.vector.tensor_scalar(uf[:], t_f, 1.0, 0.5, OP.add, OP.mult)
                ui = tmp.tile([128, KH, BC], F32, tag="ui")
                nc.vector.tensor_scalar(ui[:], t_i, 1.0, 0.5, OP.add, OP.mult)
                m1 = tmp.tile([128, KH, BC], F32, tag="m1")
                nc.vector.tensor_tensor(m1[:], uf[:], c_prev[:], OP.mult)
                m2 = tmp.tile([128, KH, BC], F32, tag="m2")
                nc.vector.tensor_tensor(m2[:], ui[:], t_g, OP.mult)
                c_new = state.tile([128, KH, BC], F32, tag="c")
                nc.vector.tensor_tensor(c_new[:], m1[:], m2[:], OP.add)
                tc_t = tmp.tile([128, KH, BC], F16, tag="tct")
                nc.scalar.activation(tc_t[:], c_new[:], AF.Tanh)
                uo = tmp.tile([128, KH, BC], F16, tag="uo")
                nc.vector.tensor_scalar(uo[:], t_o, 1.0, 0.5, OP.add, OP.mult)
                h_new = state.tile([128, KH, BC], F16, tag="h")
                nc.vector.tensor_tensor(h_new[:], uo[:], tc_t[:], OP.mult)
                nc.sync.dma_start(out_d.ap()[t], h_new[:])
                h_prev, c_prev = h_new, c_new

            for c in range(NCHUNK):
                xgc = phase_a(c)
                for tl in range(TC):
                    scan_step(c * TC + tl, xgc, tl)

    nc.compile()
    return nc


_CACHED_NC = None


def _get_nc():
    global _CACHED_NC
    if _CACHED_NC is None:
        _CACHED_NC = build_nc()
    return _CACHED_NC


def _prep_inputs(video_feats, W_e, b_e, W_ih1, W_hh1, b_ih1, b_hh1,
                 W_ih2, W_hh2, b_ih2, b_hh2):
    """Build the 8 per-core input maps (host-side shard + layout prep)."""
    # gate scaling: i, f, o rows get 0.5 (sigmoid-via-tanh); g rows 1.0
    s = np.ones((4 * H,), np.float32)
    s[0 * H : 2 * H] = 0.5
    s[3 * H : 4 * H] = 0.5

    wet = np.ascontiguousarray(W_e.T).astype(np.float16).reshape(KF, 128, P)
    bet = np.ascontiguousarray(b_e.reshape(KP, 128).T).astype(np.float32)

    per_dir = []
    for (W_ih, W_hh, b_ih, b_hh) in (
        (W_ih1, W_hh1, b_ih1, b_hh1),
        (W_ih2, W_hh2, b_ih2, b_hh2),
    ):
        wih = np.ascontiguousarray((W_ih * s[:, None]).T).astype(np.float16)
        whh = np.ascontiguousarray((W_hh * s[:, None]).T).astype(np.float16)
        bb = ((b_ih + b_hh) * s).astype(np.float32)
        per_dir.append(
            (
                wih.reshape(KP, 128, 4 * H),
                whh.reshape(KH, 128, 4 * H),
                np.ascontiguousarray(bb.reshape(MG, 128).T),
            )
        )

    # videoT [F, T, B] fp16
    vt_full = np.ascontiguousarray(video_feats.transpose(2, 1, 0)).astype(np.float16)
    vt_rev = np.ascontiguousarray(vt_full[:, ::-1, :])

    in_maps = []
    for core in range(8):
        g, d = divmod(core, 2)
        src = vt_full if d == 0 else vt_rev
        vt = np.ascontiguousarray(src[:, :, g * BC : (g + 1) * BC]).reshape(
            KF, 128, T, BC
        )
        wih, whh, bb = per_dir[d]
        in_maps.append(
            {
                "vt": vt,
                "w_et": wet,
                "b_e_t": bet,
                "w_iht": wih,
                "w_hht": whh,
                "bias": bb,
            }
        )
    return in_maps


last_exec_ns = None
last_wall_s = None


def kernel(**inputs):
    global last_exec_ns, last_wall_s
    nc = _get_nc()
    in_maps = _prep_inputs(**inputs)
    t0 = time.perf_counter()
    res = run_bass_kernel_spmd(nc, in_maps, core_ids=list(range(8)))
    last_wall_s = time.perf_counter() - t0
    last_exec_ns = res.exec_time_ns

    lstm1 = np.empty((B, T, H), np.float32)
    lstm2 = np.empty((B, T, H), np.float32)
    for core in range(8):
        g, d = divmod(core, 2)
        oh = res.results[core]["out_h"]  # [T, 128, KH, BC] f16
        h = np.transpose(oh.astype(np.float32), (3, 0, 2, 1)).reshape(BC, T, H)
        if d == 0:
            lstm1[g * BC : (g + 1) * BC] = h
        else:
            lstm2[g * BC : (g + 1) * BC] = h[:, ::-1, :]
    return (lstm1, lstm2)


# revision 16
# speedup vs baseline: 1.0112x; 1.0112x over previous
"""BiEncoder (bidirectional LSTM over video features) Trainium2 kernel.

Sharding: 8 NeuronCores = 4 batch groups (B=64 each) x 2 directions.
Every core runs the SAME program (SPMD); the host hands backward-direction
cores time-reversed inputs and the direction's weights, and un-reverses the
outputs.

Per-core program:
  phase A (per 8-step chunk): embed  v = video @ W_e.T + b_e   (fp16 matmul)
                              xg     = v @ W_ih_s.T + b_s      (fp16 matmul)
  phase B (scan, 64 steps):   hg     = W_hh_s @ h_prev         (fp16 matmul)
                              t      = tanh(hg + xg)           (one ACT op)
                              c      = (t_f+1)/2*c + (t_i+1)/2*t_g
                              h      = (t_o+1)/2*tanh(c)
  using sigmoid(x) = (tanh(x/2)+1)/2 with the 1/2 folded into the i/f/o
  rows of W_ih/W_hh/bias on the host, so ONE tanh instruction covers all
  four gate groups.
"""

import sys
import time

for _p in ("/opt/trn_rl_repo", "/root/.axon_site/_ro/trn_rl_repo"):
    if _p not in sys.path:
        sys.path.insert(0, _p)

import numpy as np

import concourse.tile as tile
from concourse import bacc, mybir
from concourse.bass import ts
from concourse.bass_utils import run_bass_kernel_spmd

F16 = mybir.dt.float16
F32 = mybir.dt.float32
AF = mybir.ActivationFunctionType
OP = mybir.AluOpType

B, T, F, P, H = 256, 64, 2048, 512, 512
NB = 4          # batch groups
BC = B // NB    # 64 per-core batch
TC = 8          # timesteps per phase-A chunk
NCHUNK = T // TC
KF = F // 128   # 16  F tiles
KP = P // 128   # 4   P tiles
KH = H // 128   # 4   H tiles
MG = 4 * H // 128  # 16 gate tiles


def build_nc():
    nc = bacc.Bacc("TRN2", target_bir_lowering=False, debug=False, num_devices=8)

    # all layouts partition-major so every DMA is one long contiguous run
    # per partition (minimizes DMA descriptor count)
    vt_d = nc.dram_tensor("vt", [NCHUNK, 128, KF, TC, BC], F16, kind="ExternalInput")
    wet_d = nc.dram_tensor("w_et", [128, KF, P], F16, kind="ExternalInput")
    bet_d = nc.dram_tensor("b_e_t", [128, KP], F32, kind="ExternalInput")
    wih_d = nc.dram_tensor("w_iht", [128, KP, 4 * H], F16, kind="ExternalInput")
    whh_d = nc.dram_tensor("w_hht", [128, KH, 4 * H], F16, kind="ExternalInput")
    bias_d = nc.dram_tensor("bias", [128, MG], F32, kind="ExternalInput")
    out_d = nc.dram_tensor("out_h", [NCHUNK, 128, TC, KH, BC], F16, kind="ExternalOutput")

    with tile.TileContext(nc) as tc:
        with (
            tc.tile_pool(name="const", bufs=1) as const,
            tc.tile_pool(name="vload", bufs=2) as vload,
            tc.tile_pool(name="vtp", bufs=2) as vtp,
            tc.tile_pool(name="xchunk", bufs=2) as xchunk,
            tc.tile_pool(name="state", bufs=3) as state,
            tc.tile_pool(name="tmp", bufs=2) as tmp,
            tc.tile_pool(name="psv", bufs=2, space="PSUM") as psv,
            tc.tile_pool(name="psx", bufs=2, space="PSUM") as psx,
            tc.tile_pool(name="psg", bufs=2, space="PSUM") as psg,
        ):
            # resident weights
            wet = const.tile([128, KF, P], F16)
            nc.sync.dma_start(wet[:], wet_d.ap())
            wih = const.tile([128, KP, 4 * H], F16)
            nc.sync.dma_start(wih[:], wih_d.ap())
            whh = const.tile([128, KH, 4 * H], F16)
            nc.sync.dma_start(whh[:], whh_d.ap())
            bet = const.tile([128, KP], F32)
            nc.sync.dma_start(bet[:], bet_d.ap())
            bias = const.tile([128, MG], F32)
            nc.sync.dma_start(bias[:], bias_d.ap())

            # initial state
            h_prev = state.tile([128, KH, BC], F16, tag="h")
            nc.gpsimd.memset(h_prev[:], 0.0)
            c_prev = state.tile([128, KH, BC], F32, tag="c")
            nc.gpsimd.memset(c_prev[:], 0.0)

            xg_chunks = []

            def phase_a(c):
                vch = vload.tile([128, KF, TC * BC], F16, tag="vch")
                nc.sync.dma_start(
                    vch[:], vt_d.ap()[c].rearrange("p ko t b -> p ko (t b)")
                )
                vsb = vtp.tile([128, KP, TC * BC], F16, tag="vsb")
                for mp in range(KP):
                    pv = psv.tile([128, TC * BC], F32, tag="pv")
                    for ko in range(KF):
                        nc.tensor.matmul(
                            pv[:],
                            wet[:, ko, ts(mp, 128)],
                            vch[:, ko, :],
                            start=(ko == 0),
                            stop=(ko == KF - 1),
                        )
                    # v = psum + b_e   (fp16 out; on ACT — idle during phase A)
                    nc.scalar.activation(
                        vsb[:, mp, :], pv[:], AF.Identity, bias=bet[:, mp : mp + 1]
                    )
                xgc = xchunk.tile([128, TC, MG, BC], F16, tag="xgc")
                for m in range(MG):
                    px = psx.tile([128, TC, BC], F32, tag="px")
                    for kp in range(KP):
                        nc.tensor.matmul(
                            px[:],
                            wih[:, kp, ts(m, 128)],
                            vsb[:, kp, :],
                            start=(kp == 0),
                            stop=(kp == KP - 1),
                        )
                    # xg = psum + bias  (fp16 out, strided over t; on ACT)
                    nc.scalar.activation(
                        xgc[:, :, m, :], px[:], AF.Identity, bias=bias[:, m : m + 1]
                    )
                return xgc

            def scan_step(t, xgc, tl, hstage):
                nonlocal h_prev, c_prev
                pg = psg.tile([128, MG, BC], F32, tag="pg")
                th = tmp.tile([128, MG, BC], F16, tag="th")
                # two m-halves so DVE-add/ACT-tanh of half 0 overlap the PE
                # matmuls of half 1
                HM = MG // 2
                for half in range(2):
                    for m in range(half * HM, (half + 1) * HM):
                        for kh in range(KH):
                            nc.tensor.matmul(
                                pg[:, m, :],
                                whh[:, kh, ts(m, 128)],
                                h_prev[:, kh, :],
                                start=(kh == 0),
                                stop=(kh == KH - 1),
                            )
                    sl = slice(half * HM, (half + 1) * HM)
                    gs = tmp.tile([128, HM, BC], F16, tag=f"gs{half}")
                    nc.vector.tensor_tensor(gs[:], pg[:, sl, :], xgc[:, tl, sl, :], OP.add)
                    nc.scalar.activation(th[:, sl, :], gs[:], AF.Tanh)
                t_i = th[:, 0 * KH : 1 * KH, :]
                t_g = th[:, 2 * KH : 3 * KH, :]
                t_o = th[:, 3 * KH : 4 * KH, :]
                # u_if = (t_[i,f] + 1) * 0.5 in one op
                uif = tmp.tile([128, 2 * KH, BC], F32, tag="uif")
                nc.vector.tensor_scalar(uif[:], th[:, 0 : 2 * KH, :], 1.0, 0.5, OP.add, OP.mult)
                m1 = tmp.tile([128, KH, BC], F32, tag="m1")
                nc.vector.tensor_tensor(m1[:], uif[:, KH:, :], c_prev[:], OP.mult)
                m2 = tmp.tile([128, KH, BC], F32, tag="m2")
                nc.vector.tensor_tensor(m2[:], uif[:, :KH, :], t_g, OP.mult)
                c_new = state.tile([128, KH, BC], F32, tag="c")
                nc.vector.tensor_tensor(c_new[:], m1[:], m2[:], OP.add)
                tc_t = tmp.tile([128, KH, BC], F16, tag="tct")
                nc.scalar.activation(tc_t[:], c_new[:], AF.Tanh)
                uo = tmp.tile([128, KH, BC], F16, tag="uo")
                nc.vector.tensor_scalar(uo[:], t_o, 1.0, 0.5, OP.add, OP.mult)
                h_new = hstage[:, tl, :, :]
                nc.vector.tensor_tensor(h_new, uo[:], tc_t[:], OP.mult)
                h_prev, c_prev = h_new, c_new

            for c in range(NCHUNK):
                xgc = phase_a(c)
                hstage = state.tile([128, TC, KH, BC], F16, tag="hs")
                for tl in range(TC):
                    scan_step(c * TC + tl, xgc, tl, hstage)
                nc.sync.dma_start(out_d.ap()[c], hstage[:])

    nc.compile()
    return nc


_CACHED_NC = None


def _get_nc():
    global _CACHED_NC
    if _CACHED_NC is None:
        _CACHED_NC = build_nc()
    return _CACHED_NC


def _prep_inputs(video_feats, W_e, b_e, W_ih1, W_hh1, b_ih1, b_hh1,
                 W_ih2, W_hh2, b_ih2, b_hh2):
    """Build the 8 per-core input maps (host-side shard + layout prep)."""
    # gate scaling: i, f, o rows get 0.5 (sigmoid-via-tanh); g rows 1.0
    s = np.ones((4 * H,), np.float32)
    s[0 * H : 2 * H] = 0.5
    s[3 * H : 4 * H] = 0.5

    wet = np.ascontiguousarray(
        W_e.T.astype(np.float16).reshape(KF, 128, P).transpose(1, 0, 2)
    )
    bet = np.ascontiguousarray(b_e.reshape(KP, 128).T).astype(np.float32)

    per_dir = []
    for (W_ih, W_hh, b_ih, b_hh) in (
        (W_ih1, W_hh1, b_ih1, b_hh1),
        (W_ih2, W_hh2, b_ih2, b_hh2),
    ):
        wih = (W_ih * s[:, None]).T.astype(np.float16)
        whh = (W_hh * s[:, None]).T.astype(np.float16)
        bb = ((b_ih + b_hh) * s).astype(np.float32)
        per_dir.append(
            (
                np.ascontiguousarray(wih.reshape(KP, 128, 4 * H).transpose(1, 0, 2)),
                np.ascontiguousarray(whh.reshape(KH, 128, 4 * H).transpose(1, 0, 2)),
                np.ascontiguousarray(bb.reshape(MG, 128).T),
            )
        )

    # videoT [F, T, B] fp16
    vt_full = np.ascontiguousarray(video_feats.transpose(2, 1, 0)).astype(np.float16)
    vt_rev = np.ascontiguousarray(vt_full[:, ::-1, :])

    in_maps = []
    for core in range(8):
        g, d = divmod(core, 2)
        src = vt_full if d == 0 else vt_rev
        # [F,T,Bc] -> [NCHUNK, 128, KF, TC, BC]
        vt = np.ascontiguousarray(
            src[:, :, g * BC : (g + 1) * BC]
            .reshape(KF, 128, NCHUNK, TC, BC)
            .transpose(2, 1, 0, 3, 4)
        )
        wih, whh, bb = per_dir[d]
        in_maps.append(
            {
                "vt": vt,
                "w_et": wet,
                "b_e_t": bet,
                "w_iht": wih,
                "w_hht": whh,
                "bias": bb,
            }
        )
    return in_maps


last_exec_ns = None
last_wall_s = None


def kernel(**inputs):
    global last_exec_ns, last_wall_s
    nc = _get_nc()
    in_maps = _prep_inputs(**inputs)
    t0 = time.perf_counter()
    res = run_bass_kernel_spmd(nc, in_maps, core_ids=list(range(8)))
    last_wall_s = time.perf_counter() - t0
    last_exec_ns = res.exec_time_ns

    lstm1 = np.empty((B, T, H), np.float32)
    lstm2 = np.empty((B, T, H), np.float32)
    for core in range(8):
        g, d = divmod(core, 2)
        oh = res.results[core]["out_h"]  # [NCHUNK, 128, TC, KH, BC] f16
        h = np.transpose(oh.astype(np.float32), (4, 0, 2, 3, 1)).reshape(BC, T, H)
        if d == 0:
            lstm1[g * BC : (g + 1) * BC] = h
        else:
            lstm2[g * BC : (g + 1) * BC] = h[:, ::-1, :]
    return (lstm1, lstm2)
